# revision 7
# baseline (speedup 1.0000x reference)
"""HGT message-passing kernel for 8 Trainium2 NeuronCores.

Strategy:
- Disease nodes are pure sinks (never sources, not in the output), so the
  disease pathway and edge types dd/pd are dead code: only drug/protein
  nodes and edge types drdr (d->d), dp (d->p), pp (p->p) are computed.
- Nodes dst-sharded across the 8 cores. Edges sorted by dst into 128-node
  groups; each group's edges processed in 128-edge tiles.
- Relation transforms folded into weights on host: a_rel/p_rel fold into a
  Q-side block-diagonal matrix, m_rel applied post-aggregation.
- Per edge tile: indirect-DMA gather of interleaved [k|v] rows (1KB/row),
  selection-matrix (is_equal vs iota) matmuls implement per-dst-segment
  softmax sums; numerator and denominator accumulate in PSUM per group.
- KV tables AllGathered between layers.
"""
import numpy as np

P = 128
HEADS, D = 8, 16
ND, NP = 30000, 40000          # drug / protein node counts
NDp, NPp = 3840, 5120          # padded per-core shard sizes (30/40 tiles)
SD, SP = ND // 8, NP // 8      # exact per-core shard sizes
ROWS = NDp + NPp               # per-core kv rows (padded)
G_D, G_P = NDp // P, NPp // P  # dst groups per core
IN_DIM, HID = 732, 128
NC_ = 8

_cache = {}
timed_ns = None
last_ck = None
last_in_maps = None


def _prep_edges(src, dst, n_dst_shard, groups):
    """Shard by dst, sort, group into 128-dst groups, pad to uniform tiles.

    Returns srcs [P, TT], dstl [P, TT] int32 per core and group tile counts.
    """
    per_core = []
    for c in range(NC_):
        m = (dst >= c * n_dst_shard) & (dst < (c + 1) * n_dst_shard)
        s, d = src[m], dst[m] - c * n_dst_shard
        o = np.argsort(d, kind="stable")
        per_core.append((s[o], d[o]))
    T = np.ones(groups, np.int64)
    counts = np.zeros((NC_, groups), np.int64)
    for c in range(NC_):
        _, d = per_core[c]
        g = d // P
        cnt = np.bincount(g, minlength=groups)
        counts[c] = cnt
        T = np.maximum(T, (cnt + P - 1) // P)
    TT = int(T.sum())
    srcs = np.zeros((NC_, P, TT), np.int32)
    dstl = np.full((NC_, P, TT), 255, np.int32)
    for c in range(NC_):
        s, d = per_core[c]
        g = d // P
        start = 0
        toff = 0
        for gi in range(groups):
            n = int(counts[c, gi])
            se, de = s[start : start + n], d[start : start + n] - gi * P
            pad = int(T[gi]) * P - n
            se = np.concatenate([se, np.zeros(pad, s.dtype)])
            de = np.concatenate([de, np.full(pad, 255, d.dtype)])
            srcs[c, :, toff : toff + int(T[gi])] = se.reshape(int(T[gi]), P).T
            dstl[c, :, toff : toff + int(T[gi])] = de.reshape(int(T[gi]), P).T
            start += n
            toff += int(T[gi])
    return srcs, dstl, [int(x) for x in T]


def _remap_drug(n):
    return (n // SD) * ROWS + n % SD


def _remap_prot(n):
    return (n // SP) * ROWS + NDp + n % SP


def _bd(blocks):
    """blockdiag of 8 [16,16] blocks -> [128,128]."""
    out = np.zeros((HID, HID), np.float32)
    for h in range(HEADS):
        out[h * D : (h + 1) * D, h * D : (h + 1) * D] = blocks[h]
    return out


def _bcast(v, w):
    return np.tile(np.asarray(v, np.float32).reshape(1, w), (P, 1))


def _build(T_tables, host):
    import concourse.bacc as bacc
    import concourse.bass as bass
    import concourse.tile as tile
    from concourse import mybir
    from concourse.masks import make_identity

    FT, IT = mybir.dt.float32, mybir.dt.int32
    nc = bacc.Bacc(None)

    ins = {}
    for name, arr in host.items():
        dt_ = FT if np.issubdtype(arr.dtype, np.floating) else IT
        ins[name] = nc.declare_dram_parameter(name, list(arr.shape), dt_, isOutput=False)
    out_d = nc.declare_dram_parameter("out_d", [NDp, 1], FT, isOutput=True)
    out_p = nc.declare_dram_parameter("out_p", [NPp, 1], FT, isOutput=True)

    ETYPES = [("drdr", "d", "d"), ("dp", "d", "p"), ("pp", "p", "p")]

    with tile.TileContext(nc) as tc:
        with (
            tc.tile_pool(name="const", bufs=1) as cp,
            tc.tile_pool(name="sbuf", bufs=4) as pool,
            tc.tile_pool(name="acc", bufs=2) as accp,
            tc.tile_pool(name="psum", bufs=3, space="PSUM") as psum,
            tc.tile_pool(name="psg", bufs=2, space="PSUM") as psg,
            tc.tile_pool(name="dram", bufs=1, space="DRAM") as dram,
        ):
            ident = cp.tile([P, P], FT)
            make_identity(nc, ident[:])
            iota = cp.tile([P, P], IT)
            nc.gpsimd.iota(iota[:], pattern=[[1, P]], base=0, channel_multiplier=0)

            def cload(name, w=P):
                t = cp.tile([P, w], FT, tag=name)
                nc.sync.dma_start(out=t[:], in_=ins[name][:])
                return t

            consts = {}
            for l in range(2):
                for t in "dp":
                    for nm in (f"wkv{l}{t}", f"bkv{l}{t}"):
                        consts[nm] = cload(nm, 256)
                    for nm in (f"wa{l}{t}", f"ba{l}{t}"):
                        consts[nm] = cload(nm)
                for r, _, _ in ETYPES:
                    for nm in (f"wq{l}{r}", f"bq{l}{r}", f"bdm{l}{r}"):
                        consts[nm] = cload(nm)
            for t in "dp":
                consts[f"blin{t}"] = cload(f"blin{t}")
                for c6 in range(6):
                    consts[f"wlin{t}{c6}"] = cload(f"wlin{t}{c6}")
            consts["wout"] = cload("wout")

            # internal DRAM
            kv_own = [dram.tile([ROWS, 256], FT, name=f"kvown{l}", tag=f"kvown{l}") for l in range(2)]
            kv_all = [dram.tile([ROWS * NC_, 256], FT, name=f"kvall{l}", tag=f"kvall{l}") for l in range(2)]
            hprev = [dram.tile([ROWS, HID], FT, name=f"hprev{l}", tag=f"hprev{l}") for l in range(2)]
            qrel = {(l, r): dram.tile([NDp if dt == "d" else NPp, HID], FT, name=f"qrel{l}{r}", tag=f"qrel{l}{r}")
                    for l in range(2) for r, _, dt in ETYPES}

            def transpose_to_sbuf(src_sb, w=P, eng="v"):
                ps = psum.tile([P, P], FT, space="PSUM", tag="psB")
                nc.tensor.transpose(out=ps[:w, :P], in_=src_sb[:, :w], identity=ident[:])
                sb = pool.tile([P, P], FT, tag="trsb")
                if eng == "v":
                    nc.vector.tensor_copy(out=sb[:w, :], in_=ps[:w, :])
                else:
                    nc.scalar.copy(out=sb[:w, :], in_=ps[:w, :])
                return sb

            def produce_tables(h_sb, l, t, row0, ti):
                """From node-major h tile, write kv/qrel/hprev tables of conv layer l."""
                hT = transpose_to_sbuf(h_sb)
                kv_ps = psum.tile([P, 256], FT, space="PSUM", tag="ps")
                nc.tensor.matmul(kv_ps[:], lhsT=hT[:], rhs=consts[f"wkv{l}{t}"][:], start=True, stop=True)
                kv_sb = pool.tile([P, 256], FT, tag="kvsb")
                nc.vector.tensor_tensor(out=kv_sb[:], in0=kv_ps[:], in1=consts[f"bkv{l}{t}"][:], op=mybir.AluOpType.add)
                nc.sync.dma_start(out=kv_own[l][row0 + ti * P : row0 + (ti + 1) * P, :], in_=kv_sb[:])
                for r, _, dt in ETYPES:
                    if dt != t:
                        continue
                    q_ps = psum.tile([P, P], FT, space="PSUM", tag="ps")
                    nc.tensor.matmul(q_ps[:], lhsT=hT[:], rhs=consts[f"wq{l}{r}"][:], start=True, stop=True)
                    q_sb = pool.tile([P, P], FT, tag="qsb")
                    nc.vector.tensor_tensor(out=q_sb[:], in0=q_ps[:], in1=consts[f"bq{l}{r}"][:], op=mybir.AluOpType.add)
                    nc.sync.dma_start(out=qrel[(l, r)][ti * P : (ti + 1) * P, :], in_=q_sb[:])
                nc.sync.dma_start(out=hprev[l][row0 + ti * P : row0 + (ti + 1) * P, :], in_=h_sb[:])

            # ---------------- PHASE LIN ----------------
            for t, ntiles, row0 in (("d", G_D, 0), ("p", G_P, NDp)):
                for ti in range(ntiles):
                    xt = pool.tile([P, IN_DIM], FT, tag="xt")
                    nc.sync.dma_start(out=xt[:], in_=ins[f"x{t}"][ti * P : (ti + 1) * P, :])
                    h_ps = psum.tile([P, P], FT, space="PSUM", tag="ps")
                    for c6 in range(6):
                        w = min(P, IN_DIM - c6 * P)
                        xT = transpose_to_sbuf(xt[:, c6 * P : c6 * P + w], w)
                        nc.tensor.matmul(h_ps[:], lhsT=xT[:w, :], rhs=consts[f"wlin{t}{c6}"][:w, :],
                                         start=(c6 == 0), stop=(c6 == 5))
                    hb = pool.tile([P, P], FT, tag="hb")
                    nc.vector.tensor_tensor(out=hb[:], in0=h_ps[:], in1=consts[f"blin{t}"][:], op=mybir.AluOpType.add)
                    h0 = pool.tile([P, P], FT, tag="h0")
                    nc.scalar.activation(out=h0[:], in_=hb[:], func=mybir.ActivationFunctionType.Relu)
                    produce_tables(h0, 0, t, row0, ti)

            def allgather(l):
                nc.gpsimd.collective_compute(
                    "AllGather", mybir.AluOpType.bypass,
                    replica_groups=[list(range(NC_))],
                    ins=[kv_own[l][:].opt()], outs=[kv_all[l][:].opt()],
                )

            allgather(0)

            # ---------------- CONV LAYERS ----------------
            def conv_group(l, r, g, T):
                """Process one (edge type, dst group); return contrib psum tile."""
                q_t = pool.tile([P, P], FT, tag="qt")
                nc.sync.dma_start(out=q_t[:], in_=qrel[(l, r)][g * P : (g + 1) * P, :])
                toff = sum(T_tables[r][:g])
                T_g = T_tables[r][g]
                sidx = pool.tile([P, T_g], IT, tag="sidx")
                didx = pool.tile([P, T_g], IT, tag="didx")
                nc.sync.dma_start(out=sidx[:], in_=ins[f"srcs_{r}"][:, toff : toff + T_g])
                nc.sync.dma_start(out=didx[:], in_=ins[f"dstl_{r}"][:, toff : toff + T_g])
                gp = psg.tile([P, 136], FT, space="PSUM", tag="gp")
                for t in range(T_g):
                    kv_t = pool.tile([P, 256], FT, tag="kvt")
                    nc.gpsimd.indirect_dma_start(
                        out=kv_t[:], out_offset=None, in_=kv_all[l][:],
                        in_offset=bass.IndirectOffsetOnAxis(ap=sidx[:, t : t + 1], axis=0))
                    selT = pool.tile([P, P], FT, tag="selT")
                    nc.vector.tensor_tensor(out=selT[:], in0=didx[:, t : t + 1].to_broadcast([P, P]),
                                            in1=iota[:], op=mybir.AluOpType.is_equal)
                    sel_ps = psum.tile([P, P], FT, space="PSUM", tag="psB")
                    nc.tensor.transpose(out=sel_ps[:], in_=selT[:], identity=ident[:])
                    sel = pool.tile([P, P], FT, tag="sel")
                    nc.scalar.copy(out=sel[:], in_=sel_ps[:])
                    qe_ps = psum.tile([P, P], FT, space="PSUM", tag="ps")
                    nc.tensor.matmul(qe_ps[:], lhsT=sel[:], rhs=q_t[:], start=True, stop=True)
                    qk = pool.tile([P, P], FT, tag="qk")
                    nc.vector.tensor_tensor(out=qk[:], in0=qe_ps[:], in1=kv_t[:, :P], op=mybir.AluOpType.mult)
                    work = pool.tile([P, 136], FT, tag="work")
                    alpha = pool.tile([P, 8], FT, tag="alpha")
                    nc.vector.tensor_reduce(out=alpha[:], in_=qk[:].rearrange("p (h d) -> p h d", h=8),
                                            axis=mybir.AxisListType.X, op=mybir.AluOpType.add)
                    nc.scalar.activation(out=work[:, P : P + 8], in_=alpha[:], func=mybir.ActivationFunctionType.Exp)
                    nc.vector.tensor_tensor(
                        out=work[:, :P].rearrange("p (h d) -> p h d", h=8),
                        in0=kv_t[:, P:].rearrange("p (h d) -> p h d", h=8),
                        in1=work[:, P : P + 8].unsqueeze(-1).broadcast_to([P, 8, 16]),
                        op=mybir.AluOpType.mult)
                    nc.tensor.matmul(gp[:], lhsT=selT[:], rhs=work[:], start=(t == 0), stop=(t == T_g - 1))
                # normalize + m_rel
                den = pool.tile([P, 8], FT, tag="den")
                nc.vector.tensor_scalar_add(out=den[:], in0=gp[:, P : P + 8], scalar1=1e-16)
                nc.vector.reciprocal(out=den[:], in_=den[:])
                ratio = pool.tile([P, P], FT, tag="ratio")
                nc.vector.tensor_tensor(
                    out=ratio[:].rearrange("p (h d) -> p h d", h=8),
                    in0=gp[:, :P].rearrange("p (h d) -> p h d", h=8),
                    in1=den[:].unsqueeze(-1).broadcast_to([P, 8, 16]),
                    op=mybir.AluOpType.mult)
                rT = transpose_to_sbuf(ratio, eng="s")
                c_ps = psum.tile([P, P], FT, space="PSUM", tag="ps")
                nc.tensor.matmul(c_ps[:], lhsT=rT[:], rhs=consts[f"bdm{l}{r}"][:], start=True, stop=True)
                return c_ps

            def post_stage(l, t, g, acc_sb, row0):
                gl = pool.tile([P, P], FT, tag="gl")
                nc.scalar.activation(out=gl[:], in_=acc_sb[:], func=mybir.ActivationFunctionType.Gelu)
                glT = transpose_to_sbuf(gl)
                o_ps = psum.tile([P, P], FT, space="PSUM", tag="ps")
                nc.tensor.matmul(o_ps[:], lhsT=glT[:], rhs=consts[f"wa{l}{t}"][:], start=True, stop=True)
                hp = pool.tile([P, P], FT, tag="hp")
                nc.sync.dma_start(out=hp[:], in_=hprev[l][row0 + g * P : row0 + (g + 1) * P, :])
                ob = pool.tile([P, P], FT, tag="ob")
                nc.vector.tensor_tensor(out=ob[:], in0=o_ps[:], in1=consts[f"ba{l}{t}"][:], op=mybir.AluOpType.add)
                hp2 = pool.tile([P, P], FT, tag="hp2")
                nc.vector.tensor_scalar_mul(out=hp2[:], in0=hp[:], scalar1=float(host_meta["hscale"][l][t]))
                hnew = pool.tile([P, P], FT, tag="hnew")
                nc.vector.tensor_tensor(out=hnew[:], in0=ob[:], in1=hp2[:], op=mybir.AluOpType.add)
                return hnew

            def head(hnew, t, g):
                m1 = pool.tile([P, P], FT, tag="m1")
                nc.vector.tensor_tensor(out=m1[:], in0=hnew[:], in1=consts["wout"][:], op=mybir.AluOpType.mult)
                s1 = pool.tile([P, 1], FT, tag="s1")
                nc.vector.tensor_reduce(out=s1[:], in_=m1[:], axis=mybir.AxisListType.X, op=mybir.AluOpType.add)
                s2 = pool.tile([P, 1], FT, tag="s2")
                nc.scalar.activation(out=s2[:], in_=s1[:], func=mybir.ActivationFunctionType.Sigmoid,
                                     bias=float(host_meta["bout"]))
                o = out_d if t == "d" else out_p
                nc.sync.dma_start(out=o[g * P : (g + 1) * P, :], in_=s2[:])

            host_meta = host_meta_holder
            for l in range(2):
                # drug dst: drdr only
                for g in range(G_D):
                    c_ps = conv_group(l, "drdr", g, T_tables["drdr"])
                    acc = accp.tile([P, P], FT, tag="accd")
                    nc.vector.tensor_copy(out=acc[:], in_=c_ps[:])
                    hnew = post_stage(l, "d", g, acc, 0)
                    if l == 0:
                        produce_tables(hnew, 1, "d", 0, g)
                    else:
                        head(hnew, "d", g)
                # protein dst: dp + pp
                for g in range(G_P):
                    c1 = conv_group(l, "dp", g, T_tables["dp"])
                    acc = accp.tile([P, P], FT, tag="accp")
                    nc.vector.tensor_copy(out=acc[:], in_=c1[:])
                    c2 = conv_group(l, "pp", g, T_tables["pp"])
                    acc2 = accp.tile([P, P], FT, tag="accp2")
                    nc.vector.tensor_tensor(out=acc2[:], in0=acc[:], in1=c2[:], op=mybir.AluOpType.add)
                    hnew = post_stage(l, "p", g, acc2, NDp)
                    if l == 0:
                        produce_tables(hnew, 1, "p", NDp, g)
                    else:
                        head(hnew, "p", g)
                if l == 0:
                    allgather(1)

    if not nc.is_finalized():
        nc.finalize()
    return nc


host_meta_holder = {}


def kernel(**inputs):
    from concourse.bass_utils import run_bass_kernel_spmd

    params = inputs["params"]
    # ---- fold weights on host ----
    scale = 1.0 / np.sqrt(D)
    ETYPES = [("drdr", "drug", "drug"), ("dp", "drug", "protein"), ("pp", "protein", "protein")]
    TMAP = {"d": "drug", "p": "protein"}
    weights = {}
    host_meta_holder.clear()
    host_meta_holder["bout"] = float(np.asarray(params["out"]["b"]).reshape(-1)[0])
    host_meta_holder["hscale"] = []
    for t in "dp":
        W = np.asarray(params["lin"][TMAP[t]]["W"], np.float32)
        for c6 in range(6):
            w = min(P, IN_DIM - c6 * P)
            chunk = np.zeros((P, P), np.float32)
            chunk[:w] = W[c6 * P : c6 * P + w]
            weights[f"wlin{t}{c6}"] = chunk
        weights[f"blin{t}"] = _bcast(params["lin"][TMAP[t]]["b"], P)
    for l in range(2):
        lp = params["convs"][l]
        hs = {}
        for t in "dp":
            tt = TMAP[t]
            Wk = np.asarray(lp["k"][tt]["W"], np.float32)
            Wv = np.asarray(lp["v"][tt]["W"], np.float32)
            bk = np.asarray(lp["k"][tt]["b"], np.float32)
            bv = np.asarray(lp["v"][tt]["b"], np.float32)
            weights[f"wkv{l}{t}"] = np.concatenate([Wk, Wv], 1)
            weights[f"bkv{l}{t}"] = _bcast(np.concatenate([bk, bv]), 256)
            sk = 1.0 / (1.0 + np.exp(-float(np.asarray(lp["skip"][tt]))))
            weights[f"wa{l}{t}"] = np.asarray(lp["a"][tt]["W"], np.float32) * sk
            weights[f"ba{l}{t}"] = _bcast(np.asarray(lp["a"][tt]["b"], np.float32) * sk, P)
            hs[t] = 1.0 - sk
        host_meta_holder["hscale"].append(hs)
        for r, st, dt in ETYPES:
            A = np.asarray(lp["a_rel"][r], np.float32)
            M = np.asarray(lp["m_rel"][r], np.float32)
            p_r = np.asarray(lp["p_rel"][r], np.float32)
            BDqa = _bd([A[h].T * (p_r[h] * scale) for h in range(HEADS)])
            Wq = np.asarray(lp["q"][TMAP[dt[0]]]["W"], np.float32)
            bq = np.asarray(lp["q"][TMAP[dt[0]]]["b"], np.float32)
            weights[f"wq{l}{r}"] = Wq @ BDqa
            weights[f"bq{l}{r}"] = _bcast(bq @ BDqa, P)
            weights[f"bdm{l}{r}"] = _bd([M[h] for h in range(HEADS)])
    weights["wout"] = _bcast(np.asarray(params["out"]["W"], np.float32)[:, 0], P)
    weights = {k: np.ascontiguousarray(v, np.float32) for k, v in weights.items()}
    # ---- edges ----
    key = None
    edge_data = {}
    T_tables = {}
    for r, st, dt in ETYPES:
        src = np.asarray(inputs[f"edge_{r}_src"])
        dst = np.asarray(inputs[f"edge_{r}_dst"])
        src = _remap_drug(src) if st == "drug" else _remap_prot(src)
        nds = SD if dt == "drug" else SP
        groups = G_D if dt == "drug" else G_P
        srcs, dstl, T = _prep_edges(src, dst, nds, groups)
        edge_data[r] = (srcs, dstl)
        T_tables[r] = T

    # ---- per-core inputs ----
    xd = np.asarray(inputs["x_drug"], np.float32)
    xp = np.asarray(inputs["x_protein"], np.float32)
    in_maps = []
    for c in range(NC_):
        m = dict(weights)
        pad_d = np.zeros((NDp, IN_DIM), np.float32)
        pad_d[:SD] = xd[c * SD : (c + 1) * SD]
        pad_p = np.zeros((NPp, IN_DIM), np.float32)
        pad_p[:SP] = xp[c * SP : (c + 1) * SP]
        m["xd"] = pad_d
        m["xp"] = pad_p
        for r, _, _ in ETYPES:
            m[f"srcs_{r}"] = edge_data[r][0][c]
            m[f"dstl_{r}"] = edge_data[r][1][c]
        in_maps.append(m)

    ck = ("prog", tuple(tuple(T_tables[r]) for r in ("drdr", "dp", "pp")))
    if ck not in _cache:
        _cache[ck] = _build(T_tables, in_maps[0])
    nc = _cache[ck]
    global last_ck, last_in_maps
    last_ck = ck
    last_in_maps = in_maps
    res = run_bass_kernel_spmd(nc, in_maps, list(range(NC_)))
    od = np.concatenate([res.results[c]["out_d"][:SD] for c in range(NC_)], 0)
    op = np.concatenate([res.results[c]["out_p"][:SP] for c in range(NC_)], 0)
    return od, op


# revision 8
# speedup vs baseline: 2.0974x; 2.0974x over previous
"""HGT message-passing kernel for 8 Trainium2 NeuronCores.

Strategy:
- Disease nodes are pure sinks (never sources, not in the output), so the
  disease pathway and edge types dd/pd are dead code: only drug/protein
  nodes and edge types drdr (d->d), dp (d->p), pp (p->p) are computed.
- Nodes dst-sharded across the 8 cores. Edges sorted by dst into 128-node
  groups; each group's edges processed in 128-edge tiles.
- Relation transforms folded into weights on host: a_rel/p_rel fold into a
  Q-side block-diagonal matrix, m_rel applied post-aggregation.
- Per edge tile: indirect-DMA gather of interleaved [k|v] rows (1KB/row),
  selection-matrix (is_equal vs iota) matmuls implement per-dst-segment
  softmax sums; numerator and denominator accumulate in PSUM per group.
- KV tables AllGathered between layers.
"""
import numpy as np

P = 128
HEADS, D = 8, 16
ND, NP = 30000, 40000          # drug / protein node counts
NDp, NPp = 3840, 5120          # padded per-core shard sizes (30/40 tiles)
SD, SP = ND // 8, NP // 8      # exact per-core shard sizes
ROWS = NDp + NPp               # per-core kv rows (padded)
G_D, G_P = NDp // P, NPp // P  # dst groups per core
IN_DIM, HID = 732, 128
NC_ = 8

_cache = {}
timed_ns = None
last_ck = None
last_in_maps = None


def _prep_edges(src, dst, n_dst_shard, groups):
    """Shard by dst, sort, group into 128-dst groups, pad to uniform tiles.

    Returns srcs [P, TT], dstl [P, TT] int32 per core and group tile counts.
    """
    per_core = []
    for c in range(NC_):
        m = (dst >= c * n_dst_shard) & (dst < (c + 1) * n_dst_shard)
        s, d = src[m], dst[m] - c * n_dst_shard
        o = np.argsort(d, kind="stable")
        per_core.append((s[o], d[o]))
    T = np.ones(groups, np.int64)
    counts = np.zeros((NC_, groups), np.int64)
    for c in range(NC_):
        _, d = per_core[c]
        g = d // P
        cnt = np.bincount(g, minlength=groups)
        counts[c] = cnt
        T = np.maximum(T, (cnt + P - 1) // P)
    TT = int(T.sum())
    srcs = np.zeros((NC_, P, TT), np.int32)
    dstl = np.full((NC_, P, TT), 255, np.int32)
    for c in range(NC_):
        s, d = per_core[c]
        g = d // P
        start = 0
        toff = 0
        for gi in range(groups):
            n = int(counts[c, gi])
            se, de = s[start : start + n], d[start : start + n] - gi * P
            pad = int(T[gi]) * P - n
            se = np.concatenate([se, np.zeros(pad, s.dtype)])
            de = np.concatenate([de, np.full(pad, 255, d.dtype)])
            srcs[c, :, toff : toff + int(T[gi])] = se.reshape(int(T[gi]), P).T
            dstl[c, :, toff : toff + int(T[gi])] = de.reshape(int(T[gi]), P).T
            start += n
            toff += int(T[gi])
    return srcs, dstl, [int(x) for x in T]


def _remap_drug(n):
    return (n // SD) * ROWS + n % SD


def _remap_prot(n):
    return (n // SP) * ROWS + NDp + n % SP


def _bd(blocks):
    """blockdiag of 8 [16,16] blocks -> [128,128]."""
    out = np.zeros((HID, HID), np.float32)
    for h in range(HEADS):
        out[h * D : (h + 1) * D, h * D : (h + 1) * D] = blocks[h]
    return out


def _bcast(v, w):
    return np.tile(np.asarray(v, np.float32).reshape(1, w), (P, 1))


def _build(T_tables, host):
    import concourse.bacc as bacc
    import concourse.bass as bass
    import concourse.tile as tile
    from concourse import mybir
    from concourse.masks import make_identity

    FT, IT = mybir.dt.float32, mybir.dt.int32
    nc = bacc.Bacc(None)

    ins = {}
    for name, arr in host.items():
        dt_ = FT if np.issubdtype(arr.dtype, np.floating) else IT
        ins[name] = nc.declare_dram_parameter(name, list(arr.shape), dt_, isOutput=False)
    out_d = nc.declare_dram_parameter("out_d", [NDp, 1], FT, isOutput=True)
    out_p = nc.declare_dram_parameter("out_p", [NPp, 1], FT, isOutput=True)

    ETYPES = [("drdr", "d", "d"), ("dp", "d", "p"), ("pp", "p", "p")]

    with tile.TileContext(nc) as tc:
        with (
            tc.tile_pool(name="const", bufs=1) as cp,
            tc.tile_pool(name="sbuf", bufs=3) as pool,
            tc.tile_pool(name="acc", bufs=2) as accp,
            tc.tile_pool(name="psum", bufs=4, space="PSUM") as psum,
            tc.tile_pool(name="psg", bufs=2, space="PSUM") as psg,
            tc.tile_pool(name="dram", bufs=1, space="DRAM") as dram,
        ):
            ident = cp.tile([P, P], FT)
            make_identity(nc, ident[:])
            iota = cp.tile([P, P], IT)
            nc.gpsimd.iota(iota[:], pattern=[[1, P]], base=0, channel_multiplier=0)

            def cload(name, w=P):
                t = cp.tile([P, w], FT, tag=name)
                nc.sync.dma_start(out=t[:], in_=ins[name][:])
                return t

            consts = {}
            for l in range(2):
                for t in "dp":
                    for nm in (f"wkv{l}{t}", f"bkv{l}{t}"):
                        consts[nm] = cload(nm, 256)
                    for nm in (f"wa{l}{t}", f"ba{l}{t}"):
                        consts[nm] = cload(nm)
                for r, _, _ in ETYPES:
                    for nm in (f"wq{l}{r}", f"bq{l}{r}", f"bdm{l}{r}"):
                        consts[nm] = cload(nm)
            for t in "dp":
                consts[f"blin{t}"] = cload(f"blin{t}")
                for c6 in range(6):
                    consts[f"wlin{t}{c6}"] = cload(f"wlin{t}{c6}")
            consts["wout"] = cload("wout")

            # internal DRAM
            kv_own = [dram.tile([ROWS, 256], FT, name=f"kvown{l}", tag=f"kvown{l}") for l in range(2)]
            kv_all = [dram.tile([ROWS * NC_, 256], FT, name=f"kvall{l}", tag=f"kvall{l}") for l in range(2)]
            hprev = [dram.tile([ROWS, HID], FT, name=f"hprev{l}", tag=f"hprev{l}") for l in range(2)]
            qrel = {(l, r): dram.tile([NDp if dt == "d" else NPp, HID], FT, name=f"qrel{l}{r}", tag=f"qrel{l}{r}")
                    for l in range(2) for r, _, dt in ETYPES}

            def transpose_to_sbuf(src_sb, w=P, eng="v"):
                ps = psum.tile([P, P], FT, space="PSUM", tag="ps")
                nc.tensor.transpose(out=ps[:w, :P], in_=src_sb[:, :w], identity=ident[:])
                sb = pool.tile([P, P], FT, tag="trsb")
                if eng == "v":
                    nc.vector.tensor_copy(out=sb[:w, :], in_=ps[:w, :])
                else:
                    nc.scalar.copy(out=sb[:w, :], in_=ps[:w, :])
                return sb

            def produce_tables(h_sb, l, t, row0, ti):
                """From node-major h tile, write kv/qrel/hprev tables of conv layer l."""
                hT = transpose_to_sbuf(h_sb)
                kv_ps = psum.tile([P, 256], FT, space="PSUM", tag="ps")
                nc.tensor.matmul(kv_ps[:], lhsT=hT[:], rhs=consts[f"wkv{l}{t}"][:], start=True, stop=True)
                kv_sb = pool.tile([P, 256], FT, tag="kvsb")
                nc.vector.tensor_tensor(out=kv_sb[:], in0=kv_ps[:], in1=consts[f"bkv{l}{t}"][:], op=mybir.AluOpType.add)
                nc.sync.dma_start(out=kv_own[l][row0 + ti * P : row0 + (ti + 1) * P, :], in_=kv_sb[:])
                for r, _, dt in ETYPES:
                    if dt != t:
                        continue
                    q_ps = psum.tile([P, P], FT, space="PSUM", tag="ps")
                    nc.tensor.matmul(q_ps[:], lhsT=hT[:], rhs=consts[f"wq{l}{r}"][:], start=True, stop=True)
                    q_sb = pool.tile([P, P], FT, tag="qsb")
                    nc.vector.tensor_tensor(out=q_sb[:], in0=q_ps[:], in1=consts[f"bq{l}{r}"][:], op=mybir.AluOpType.add)
                    nc.sync.dma_start(out=qrel[(l, r)][ti * P : (ti + 1) * P, :], in_=q_sb[:])
                nc.sync.dma_start(out=hprev[l][row0 + ti * P : row0 + (ti + 1) * P, :], in_=h_sb[:])

            # ---------------- PHASE LIN ----------------
            for t, ntiles, row0 in (("d", G_D, 0), ("p", G_P, NDp)):
                for ti in range(ntiles):
                    xt = pool.tile([P, IN_DIM], FT, tag="xt")
                    nc.sync.dma_start(out=xt[:], in_=ins[f"x{t}"][ti * P : (ti + 1) * P, :])
                    h_ps = psum.tile([P, P], FT, space="PSUM", tag="ps")
                    for c6 in range(6):
                        w = min(P, IN_DIM - c6 * P)
                        xT = transpose_to_sbuf(xt[:, c6 * P : c6 * P + w], w)
                        nc.tensor.matmul(h_ps[:], lhsT=xT[:w, :], rhs=consts[f"wlin{t}{c6}"][:w, :],
                                         start=(c6 == 0), stop=(c6 == 5))
                    hb = pool.tile([P, P], FT, tag="hb")
                    nc.vector.tensor_tensor(out=hb[:], in0=h_ps[:], in1=consts[f"blin{t}"][:], op=mybir.AluOpType.add)
                    h0 = pool.tile([P, P], FT, tag="h0")
                    nc.scalar.activation(out=h0[:], in_=hb[:], func=mybir.ActivationFunctionType.Relu)
                    produce_tables(h0, 0, t, row0, ti)

            def allgather(l):
                nc.gpsimd.collective_compute(
                    "AllGather", mybir.AluOpType.bypass,
                    replica_groups=[list(range(NC_))],
                    ins=[kv_own[l][:].opt()], outs=[kv_all[l][:].opt()],
                )

            allgather(0)

            # ---------------- CONV LAYERS ----------------
            def conv_group(l, r, g, T):
                """Process one (edge type, dst group); return contrib psum tile."""
                q_t = pool.tile([P, P], FT, tag="qt")
                nc.sync.dma_start(out=q_t[:], in_=qrel[(l, r)][g * P : (g + 1) * P, :])
                toff = sum(T_tables[r][:g])
                T_g = T_tables[r][g]
                sidx = pool.tile([P, T_g], IT, tag="sidx")
                didx = pool.tile([P, T_g], IT, tag="didx")
                nc.sync.dma_start(out=sidx[:], in_=ins[f"srcs_{r}"][:, toff : toff + T_g])
                nc.sync.dma_start(out=didx[:], in_=ins[f"dstl_{r}"][:, toff : toff + T_g])
                gp = psg.tile([P, 136], FT, space="PSUM", tag="gp")
                for t in range(T_g):
                    kv_t = pool.tile([P, 256], FT, tag="kvt")
                    nc.gpsimd.indirect_dma_start(
                        out=kv_t[:], out_offset=None, in_=kv_all[l][:],
                        in_offset=bass.IndirectOffsetOnAxis(ap=sidx[:, t : t + 1], axis=0))
                    selT = pool.tile([P, P], FT, tag="selT")
                    nc.vector.tensor_tensor(out=selT[:], in0=didx[:, t : t + 1].to_broadcast([P, P]),
                                            in1=iota[:], op=mybir.AluOpType.is_equal)
                    sel_ps = psum.tile([P, P], FT, space="PSUM", tag="ps")
                    nc.tensor.transpose(out=sel_ps[:], in_=selT[:], identity=ident[:])
                    sel = pool.tile([P, P], FT, tag="sel")
                    nc.scalar.copy(out=sel[:], in_=sel_ps[:])
                    qe_ps = psum.tile([P, P], FT, space="PSUM", tag="ps")
                    nc.tensor.matmul(qe_ps[:], lhsT=sel[:], rhs=q_t[:], start=True, stop=True)
                    qk = pool.tile([P, P], FT, tag="qk")
                    nc.vector.tensor_tensor(out=qk[:], in0=qe_ps[:], in1=kv_t[:, :P], op=mybir.AluOpType.mult)
                    work = pool.tile([P, 136], FT, tag="work")
                    alpha = pool.tile([P, 8], FT, tag="alpha")
                    nc.vector.tensor_reduce(out=alpha[:], in_=qk[:].rearrange("p (h d) -> p h d", h=8),
                                            axis=mybir.AxisListType.X, op=mybir.AluOpType.add)
                    nc.scalar.activation(out=work[:, P : P + 8], in_=alpha[:], func=mybir.ActivationFunctionType.Exp)
                    nc.vector.tensor_tensor(
                        out=work[:, :P].rearrange("p (h d) -> p h d", h=8),
                        in0=kv_t[:, P:].rearrange("p (h d) -> p h d", h=8),
                        in1=work[:, P : P + 8].unsqueeze(-1).broadcast_to([P, 8, 16]),
                        op=mybir.AluOpType.mult)
                    nc.tensor.matmul(gp[:], lhsT=selT[:], rhs=work[:], start=(t == 0), stop=(t == T_g - 1))
                # normalize + m_rel
                den = pool.tile([P, 8], FT, tag="den")
                nc.vector.tensor_scalar_add(out=den[:], in0=gp[:, P : P + 8], scalar1=1e-16)
                nc.vector.reciprocal(out=den[:], in_=den[:])
                ratio = pool.tile([P, P], FT, tag="ratio")
                nc.vector.tensor_tensor(
                    out=ratio[:].rearrange("p (h d) -> p h d", h=8),
                    in0=gp[:, :P].rearrange("p (h d) -> p h d", h=8),
                    in1=den[:].unsqueeze(-1).broadcast_to([P, 8, 16]),
                    op=mybir.AluOpType.mult)
                rT = transpose_to_sbuf(ratio, eng="s")
                c_ps = psum.tile([P, P], FT, space="PSUM", tag="ps")
                nc.tensor.matmul(c_ps[:], lhsT=rT[:], rhs=consts[f"bdm{l}{r}"][:], start=True, stop=True)
                return c_ps

            def post_stage(l, t, g, acc_sb, row0):
                gl = pool.tile([P, P], FT, tag="gl")
                nc.scalar.activation(out=gl[:], in_=acc_sb[:], func=mybir.ActivationFunctionType.Gelu)
                glT = transpose_to_sbuf(gl)
                o_ps = psum.tile([P, P], FT, space="PSUM", tag="ps")
                nc.tensor.matmul(o_ps[:], lhsT=glT[:], rhs=consts[f"wa{l}{t}"][:], start=True, stop=True)
                hp = pool.tile([P, P], FT, tag="hp")
                nc.sync.dma_start(out=hp[:], in_=hprev[l][row0 + g * P : row0 + (g + 1) * P, :])
                ob = pool.tile([P, P], FT, tag="ob")
                nc.vector.tensor_tensor(out=ob[:], in0=o_ps[:], in1=consts[f"ba{l}{t}"][:], op=mybir.AluOpType.add)
                hp2 = pool.tile([P, P], FT, tag="hp2")
                nc.vector.tensor_scalar_mul(out=hp2[:], in0=hp[:], scalar1=float(host_meta["hscale"][l][t]))
                hnew = pool.tile([P, P], FT, tag="hnew")
                nc.vector.tensor_tensor(out=hnew[:], in0=ob[:], in1=hp2[:], op=mybir.AluOpType.add)
                return hnew

            def head(hnew, t, g):
                m1 = pool.tile([P, P], FT, tag="m1")
                nc.vector.tensor_tensor(out=m1[:], in0=hnew[:], in1=consts["wout"][:], op=mybir.AluOpType.mult)
                s1 = pool.tile([P, 1], FT, tag="s1")
                nc.vector.tensor_reduce(out=s1[:], in_=m1[:], axis=mybir.AxisListType.X, op=mybir.AluOpType.add)
                s2 = pool.tile([P, 1], FT, tag="s2")
                nc.scalar.activation(out=s2[:], in_=s1[:], func=mybir.ActivationFunctionType.Sigmoid,
                                     bias=float(host_meta["bout"]))
                o = out_d if t == "d" else out_p
                nc.sync.dma_start(out=o[g * P : (g + 1) * P, :], in_=s2[:])

            host_meta = host_meta_holder
            for l in range(2):
                # drug dst: drdr only
                for g in range(G_D):
                    c_ps = conv_group(l, "drdr", g, T_tables["drdr"])
                    acc = accp.tile([P, P], FT, tag="accd")
                    nc.vector.tensor_copy(out=acc[:], in_=c_ps[:])
                    hnew = post_stage(l, "d", g, acc, 0)
                    if l == 0:
                        produce_tables(hnew, 1, "d", 0, g)
                    else:
                        head(hnew, "d", g)
                # protein dst: dp + pp
                for g in range(G_P):
                    c1 = conv_group(l, "dp", g, T_tables["dp"])
                    acc = accp.tile([P, P], FT, tag="accp")
                    nc.vector.tensor_copy(out=acc[:], in_=c1[:])
                    c2 = conv_group(l, "pp", g, T_tables["pp"])
                    acc2 = accp.tile([P, P], FT, tag="accp2")
                    nc.vector.tensor_tensor(out=acc2[:], in0=acc[:], in1=c2[:], op=mybir.AluOpType.add)
                    hnew = post_stage(l, "p", g, acc2, NDp)
                    if l == 0:
                        produce_tables(hnew, 1, "p", NDp, g)
                    else:
                        head(hnew, "p", g)
                if l == 0:
                    allgather(1)

    if not nc.is_finalized():
        nc.finalize()
    return nc


host_meta_holder = {}


def kernel(**inputs):
    from concourse.bass_utils import run_bass_kernel_spmd

    params = inputs["params"]
    # ---- fold weights on host ----
    scale = 1.0 / np.sqrt(D)
    ETYPES = [("drdr", "drug", "drug"), ("dp", "drug", "protein"), ("pp", "protein", "protein")]
    TMAP = {"d": "drug", "p": "protein"}
    weights = {}
    host_meta_holder.clear()
    host_meta_holder["bout"] = float(np.asarray(params["out"]["b"]).reshape(-1)[0])
    host_meta_holder["hscale"] = []
    for t in "dp":
        W = np.asarray(params["lin"][TMAP[t]]["W"], np.float32)
        for c6 in range(6):
            w = min(P, IN_DIM - c6 * P)
            chunk = np.zeros((P, P), np.float32)
            chunk[:w] = W[c6 * P : c6 * P + w]
            weights[f"wlin{t}{c6}"] = chunk
        weights[f"blin{t}"] = _bcast(params["lin"][TMAP[t]]["b"], P)
    for l in range(2):
        lp = params["convs"][l]
        hs = {}
        for t in "dp":
            tt = TMAP[t]
            Wk = np.asarray(lp["k"][tt]["W"], np.float32)
            Wv = np.asarray(lp["v"][tt]["W"], np.float32)
            bk = np.asarray(lp["k"][tt]["b"], np.float32)
            bv = np.asarray(lp["v"][tt]["b"], np.float32)
            weights[f"wkv{l}{t}"] = np.concatenate([Wk, Wv], 1)
            weights[f"bkv{l}{t}"] = _bcast(np.concatenate([bk, bv]), 256)
            sk = 1.0 / (1.0 + np.exp(-float(np.asarray(lp["skip"][tt]))))
            weights[f"wa{l}{t}"] = np.asarray(lp["a"][tt]["W"], np.float32) * sk
            weights[f"ba{l}{t}"] = _bcast(np.asarray(lp["a"][tt]["b"], np.float32) * sk, P)
            hs[t] = 1.0 - sk
        host_meta_holder["hscale"].append(hs)
        for r, st, dt in ETYPES:
            A = np.asarray(lp["a_rel"][r], np.float32)
            M = np.asarray(lp["m_rel"][r], np.float32)
            p_r = np.asarray(lp["p_rel"][r], np.float32)
            BDqa = _bd([A[h].T * (p_r[h] * scale) for h in range(HEADS)])
            Wq = np.asarray(lp["q"][TMAP[dt[0]]]["W"], np.float32)
            bq = np.asarray(lp["q"][TMAP[dt[0]]]["b"], np.float32)
            weights[f"wq{l}{r}"] = Wq @ BDqa
            weights[f"bq{l}{r}"] = _bcast(bq @ BDqa, P)
            weights[f"bdm{l}{r}"] = _bd([M[h] for h in range(HEADS)])
    weights["wout"] = _bcast(np.asarray(params["out"]["W"], np.float32)[:, 0], P)
    weights = {k: np.ascontiguousarray(v, np.float32) for k, v in weights.items()}
    # ---- edges ----
    key = None
    edge_data = {}
    T_tables = {}
    for r, st, dt in ETYPES:
        src = np.asarray(inputs[f"edge_{r}_src"])
        dst = np.asarray(inputs[f"edge_{r}_dst"])
        src = _remap_drug(src) if st == "drug" else _remap_prot(src)
        nds = SD if dt == "drug" else SP
        groups = G_D if dt == "drug" else G_P
        srcs, dstl, T = _prep_edges(src, dst, nds, groups)
        edge_data[r] = (srcs, dstl)
        T_tables[r] = T

    # ---- per-core inputs ----
    xd = np.asarray(inputs["x_drug"], np.float32)
    xp = np.asarray(inputs["x_protein"], np.float32)
    in_maps = []
    for c in range(NC_):
        m = dict(weights)
        pad_d = np.zeros((NDp, IN_DIM), np.float32)
        pad_d[:SD] = xd[c * SD : (c + 1) * SD]
        pad_p = np.zeros((NPp, IN_DIM), np.float32)
        pad_p[:SP] = xp[c * SP : (c + 1) * SP]
        m["xd"] = pad_d
        m["xp"] = pad_p
        for r, _, _ in ETYPES:
            m[f"srcs_{r}"] = edge_data[r][0][c]
            m[f"dstl_{r}"] = edge_data[r][1][c]
        in_maps.append(m)

    ck = ("prog", tuple(tuple(T_tables[r]) for r in ("drdr", "dp", "pp")))
    if ck not in _cache:
        _cache[ck] = _build(T_tables, in_maps[0])
    nc = _cache[ck]
    global last_ck, last_in_maps
    last_ck = ck
    last_in_maps = in_maps
    res = run_bass_kernel_spmd(nc, in_maps, list(range(NC_)))
    od = np.concatenate([res.results[c]["out_d"][:SD] for c in range(NC_)], 0)
    op = np.concatenate([res.results[c]["out_p"][:SP] for c in range(NC_)], 0)
    return od, op
